# revision 3
# baseline (speedup 1.0000x reference)
"""Causal self-attention (B=2, T=2048, D=2048, H=16, RoPE + q_gain) on 8
Trainium2 NeuronCores.

Sharding: batch (2-way) x head-groups (4-way, 4 heads each) -> 8 cores.
Each core computes qkv projection for its 4 heads, causal attention, and a
partial output projection (Wproj row-sharded); partials are summed on host.

All matmuls run in float32r (full PE rate, ~12-bit mantissa rounding,
~1e-4 relative error).
"""

import math

import numpy as np

import concourse.bass as bass
import concourse.mybir as mybir
from concourse import bacc
from concourse.bass_utils import run_bass_kernel_spmd
from concourse.tile import TileContext

dt = mybir.dt
F32 = dt.float32
F32R = dt.float32r
AF = mybir.ActivationFunctionType

D_MODEL = 2048
N_HEADS = 16
D_HEAD = 128
B = 2
T = 2048
N_CORES = 8
HG = 4          # heads per core
TB = 512        # token block (matmul free dim)
NTB = T // TB   # 4
NKT = T // 128  # 16 k-tiles per head


def build_program():
    nc = bacc.Bacc("TRN2", target_bir_lowering=False, debug=False,
                   num_devices=N_CORES)

    xT = nc.dram_tensor("xT", [D_MODEL, T], F32R, kind="ExternalInput")
    wT = nc.dram_tensor("wT", [D_MODEL, 3 * HG * D_HEAD], F32R, kind="ExternalInput")
    wpT = nc.dram_tensor("wpT", [HG * D_HEAD, D_MODEL], F32R, kind="ExternalInput")
    cc = nc.dram_tensor("cc", [128, T], F32, kind="ExternalInput")
    ss = nc.dram_tensor("ss", [128, T], F32, kind="ExternalInput")
    gains = nc.dram_tensor("gains", [128, HG], F32, kind="ExternalInput")
    masks = nc.dram_tensor("masks", [128, 4 * TB], F32R, kind="ExternalInput")
    ones = nc.dram_tensor("ones", [128, 1], F32R, kind="ExternalInput")
    out = nc.dram_tensor("out", [T, D_MODEL], F32, kind="ExternalOutput")

    with TileContext(nc) as tc:
        with tc.tile_pool(name="const", bufs=1) as cpool, \
             tc.tile_pool(name="dram", bufs=1, space="DRAM") as dpool:
            cc_sb = cpool.tile([128, T], F32, tag="cc")
            nc.sync.dma_start(out=cc_sb[:], in_=cc[:])
            ss_sb = cpool.tile([128, T], F32, tag="ss")
            nc.sync.dma_start(out=ss_sb[:], in_=ss[:])
            gains_sb = cpool.tile([128, HG], F32, tag="gains")
            nc.sync.dma_start(out=gains_sb[:], in_=gains[:])
            masks_sb = cpool.tile([128, 4 * TB], F32R, tag="masks")
            nc.sync.dma_start(out=masks_sb[:], in_=masks[:])
            ones_sb = cpool.tile([128, 1], F32R, tag="ones")
            nc.sync.dma_start(out=ones_sb[:], in_=ones[:])

            # DRAM scratch for q/k (post-RoPE, [d, t] layout) and v ([t, d]).
            qk_dram = dpool.tile([2 * HG * D_HEAD, T], F32R, tag="qk")
            v_dram = dpool.tile([T, HG * D_HEAD], F32R, tag="v")

            # ---------------- Phase 1: QKV projection + RoPE ----------------
            with tc.tile_pool(name="wq", bufs=NKT) as wpool, \
                 tc.tile_pool(name="xp", bufs=20) as xpool, \
                 tc.tile_pool(name="st1", bufs=3) as st1, \
                 tc.tile_pool(name="psA", bufs=2, space="PSUM") as psA, \
                 tc.tile_pool(name="psB", bufs=2, space="PSUM") as psB:
                w_tiles = []
                for cb in range(NKT):
                    w_t = wpool.tile([128, 3 * HG * D_HEAD], F32R, tag="w")
                    nc.sync.dma_start(out=w_t[:], in_=wT[cb * 128:(cb + 1) * 128, :])
                    w_tiles.append(w_t)

                for tb in range(NTB):
                    tsl = slice(tb * TB, (tb + 1) * TB)
                    x_tiles = []
                    for cb in range(NKT):
                        x_t = xpool.tile([128, TB], F32R, tag="x")
                        nc.sync.dma_start(out=x_t[:], in_=xT[cb * 128:(cb + 1) * 128, tsl])
                        x_tiles.append(x_t)

                    # q (m=0..3) and k (m=4..7) head-dim tiles, [d, t] layout
                    for m in range(2 * HG):
                        q_ps = psA.tile([128, TB], F32, tag="qps")
                        for cb in range(NKT):
                            nc.tensor.matmul(
                                q_ps[:], w_tiles[cb][:, m * 128:(m + 1) * 128],
                                x_tiles[cb][:],
                                start=(cb == 0), stop=(cb == NKT - 1))
                        # RoPE: rows 0:64 pair with rows 64:128
                        qc = st1.tile([128, TB], F32, tag="qc")
                        nc.vector.tensor_mul(qc[:], q_ps[:], cc_sb[:, tsl])
                        qs = psB.tile([128, TB], F32, tag="qs")
                        nc.vector.tensor_mul(qs[:], q_ps[:], ss_sb[:, tsl])
                        qr = st1.tile([128, TB], F32R, tag="qr")
                        nc.vector.tensor_sub(qr[0:64, :], qc[0:64, :], qs[64:128, :])
                        nc.vector.tensor_add(qr[64:128, :], qc[64:128, :], qs[0:64, :])
                        nc.sync.dma_start(out=qk_dram[m * 128:(m + 1) * 128, tsl],
                                          in_=qr[:])

                    # v in [t, d] layout: x tiles stationary, wv moving
                    for tsub in range(TB // 128):
                        v_ps = psA.tile([128, HG * D_HEAD], F32, tag="vps")
                        for cb in range(NKT):
                            nc.tensor.matmul(
                                v_ps[:],
                                x_tiles[cb][:, tsub * 128:(tsub + 1) * 128],
                                w_tiles[cb][:, 2 * HG * D_HEAD:3 * HG * D_HEAD],
                                start=(cb == 0), stop=(cb == NKT - 1))
                        vr = st1.tile([128, HG * D_HEAD], F32R, tag="vr")
                        nc.scalar.copy(vr[:], v_ps[:])
                        r0 = tb * TB + tsub * 128
                        nc.sync.dma_start(out=v_dram[r0:r0 + 128, :], in_=vr[:])

            # ---------------- Phase 2: attention, Phase 3: projection -------
            with tc.tile_pool(name="qk2", bufs=2) as qkpool, \
                 tc.tile_pool(name="vt2", bufs=2) as vtpool, \
                 tc.tile_pool(name="ep", bufs=4) as epool, \
                 tc.tile_pool(name="sm", bufs=2) as smpool, \
                 tc.tile_pool(name="yt", bufs=1) as ytpool, \
                 tc.tile_pool(name="wp", bufs=1) as wppool, \
                 tc.tile_pool(name="ost", bufs=3) as ostage, \
                 tc.tile_pool(name="psS", bufs=3, space="PSUM") as psS, \
                 tc.tile_pool(name="psY", bufs=2, space="PSUM") as psY, \
                 tc.tile_pool(name="psD", bufs=2, space="PSUM") as psD:

                wp_tiles = []
                for h in range(HG):
                    wp_t = wppool.tile([128, D_MODEL], F32R, tag=f"wp{h}")
                    nc.sync.dma_start(out=wp_t[:], in_=wpT[h * 128:(h + 1) * 128, :])
                    wp_tiles.append(wp_t)

                yT_tiles = []
                for h in range(HG):
                    yT_tiles.append(ytpool.tile([128, T], F32R, tag=f"yT{h}", name=f"yT{h}"))

                for h in range(HG):
                    qt = qkpool.tile([128, T], F32R, tag="qt")
                    nc.sync.dma_start(out=qt[:], in_=qk_dram[h * 128:(h + 1) * 128, :])
                    kt = qkpool.tile([128, T], F32R, tag="kt")
                    nc.sync.dma_start(
                        out=kt[:],
                        in_=qk_dram[(HG + h) * 128:(HG + h + 1) * 128, :])
                    vt = vtpool.tile([128, T], F32R, tag="vt")
                    for kb in range(NKT):
                        nc.sync.dma_start(
                            out=vt[:, kb * 128:(kb + 1) * 128],
                            in_=v_dram[kb * 128:(kb + 1) * 128,
                                       h * 128:(h + 1) * 128])

                    for qb in range(NTB):
                        qsl = slice(qb * TB, (qb + 1) * TB)
                        y_ps = psY.tile([128, TB], F32, tag="y")
                        d_ps = psD.tile([1, TB], F32, tag="d")
                        nkb = 4 * qb + 4
                        for kb in range(nkb):
                            s_ps = psS.tile([128, TB], F32, tag="s")
                            nc.tensor.matmul(s_ps[:], kt[:, kb * 128:(kb + 1) * 128],
                                             qt[:, qsl], start=True, stop=True)
                            e_sb = epool.tile([128, TB], F32R, tag="e")
                            nc.scalar.activation(e_sb[:], s_ps[:], AF.Exp,
                                                 scale=gains_sb[:, h:h + 1])
                            r = kb - 4 * qb
                            if r >= 0:
                                nc.vector.tensor_mul(
                                    e_sb[:], e_sb[:],
                                    masks_sb[:, r * TB:(r + 1) * TB])
                            nc.tensor.matmul(y_ps[:], vt[:, kb * 128:(kb + 1) * 128],
                                             e_sb[:],
                                             start=(kb == 0), stop=(kb == nkb - 1))
                            nc.tensor.matmul(d_ps[:], ones_sb[:], e_sb[:],
                                             start=(kb == 0), stop=(kb == nkb - 1))
                        rec = smpool.tile([1, TB], F32, tag="rec")
                        nc.vector.reciprocal(rec[:], d_ps[:])
                        recb = smpool.tile([128, TB], F32, tag="recb")
                        nc.gpsimd.partition_broadcast(recb[:], rec[:])
                        nc.vector.tensor_mul(yT_tiles[h][:, qsl], y_ps[:], recb[:])

                # Phase 3: out[t, o] partial = sum_hd yT[hd, t] * wpT[hd, o]
                for mt in range(NKT):
                    msl = slice(mt * 128, (mt + 1) * 128)
                    for ob in range(NTB):
                        osl = slice(ob * TB, (ob + 1) * TB)
                        p_ps = psS.tile([128, TB], F32, tag="s", name="p_ps")
                        for h in range(HG):
                            nc.tensor.matmul(p_ps[:], yT_tiles[h][:, msl],
                                             wp_tiles[h][:, osl],
                                             start=(h == 0), stop=(h == HG - 1))
                        o_sb = ostage.tile([128, TB], F32, tag="o")
                        nc.scalar.copy(o_sb[:], p_ps[:])
                        nc.sync.dma_start(out=out[msl, osl], in_=o_sb[:])

    nc.compile()
    return nc


def prepare_core_inputs(x, Wqkv, Wproj, q_gain, rope_cos, rope_sin):
    x = np.asarray(x, dtype=np.float32)
    Wqkv = np.asarray(Wqkv, dtype=np.float32)
    Wproj = np.asarray(Wproj, dtype=np.float32)
    q_gain = np.asarray(q_gain, dtype=np.float32)
    rope_cos = np.asarray(rope_cos, dtype=np.float32)
    rope_sin = np.asarray(rope_sin, dtype=np.float32)

    cosT = np.ascontiguousarray(rope_cos.T)  # [64, T]
    sinT = np.ascontiguousarray(rope_sin.T)
    cc = np.concatenate([cosT, cosT], axis=0)  # [128, T]
    ss = np.concatenate([sinT, sinT], axis=0)

    masks = np.zeros((128, 4 * TB), dtype=np.float32)
    kk = np.arange(128)[:, None]
    qq = np.arange(TB)[None, :]
    for r in range(4):
        masks[:, r * TB:(r + 1) * TB] = ((r * 128 + kk) <= qq)

    ones = np.ones((128, 1), dtype=np.float32)
    scale = 1.0 / math.sqrt(D_HEAD)

    xT_b = [np.ascontiguousarray(x[b].T) for b in range(B)]  # [C, T]
    in_maps = []
    for c in range(N_CORES):
        b = c // HG
        hg = c % HG
        r0 = 512 * hg
        wsel = np.concatenate([
            Wqkv[r0:r0 + 512],                       # q rows for 4 heads
            Wqkv[D_MODEL + r0:D_MODEL + r0 + 512],   # k rows
            Wqkv[2 * D_MODEL + r0:2 * D_MODEL + r0 + 512],  # v rows
        ], axis=0)                                    # [1536, C]
        wT = np.ascontiguousarray(wsel.T)             # [C, 1536]
        wpT = np.ascontiguousarray(Wproj[:, r0:r0 + 512].T)  # [512, C]
        g = (q_gain[4 * hg:4 * hg + 4] * scale).astype(np.float32)
        gains = np.ascontiguousarray(np.broadcast_to(g[None, :], (128, HG)))
        in_maps.append({
            "xT": xT_b[b], "wT": wT, "wpT": wpT, "cc": cc, "ss": ss,
            "gains": gains, "masks": masks, "ones": ones,
        })
    return in_maps


_NC_CACHE = []


def kernel(x, Wqkv, Wproj, q_gain, rope_cos, rope_sin):
    if not _NC_CACHE:
        _NC_CACHE.append(build_program())
    nc = _NC_CACHE[0]
    in_maps = prepare_core_inputs(x, Wqkv, Wproj, q_gain, rope_cos, rope_sin)
    res = run_bass_kernel_spmd(nc, in_maps, list(range(N_CORES)))
    out = np.zeros((B, T, D_MODEL), dtype=np.float32)
    for c in range(N_CORES):
        out[c // HG] += res.results[c]["out"]
    return out


# revision 13
# speedup vs baseline: 10.7486x; 10.7486x over previous
"""Causal self-attention (B=2, T=2048, D=2048, H=16, RoPE + q_gain) on 8
Trainium2 NeuronCores.

Sharding: batch (2-way) x head-groups (4-way, 4 heads each) -> 8 cores.
Each core computes qkv projection for its 4 heads, causal attention, and a
partial output projection (Wproj row-sharded); partials are summed on host.

All matmuls run in float32r (full PE rate, ~12-bit mantissa rounding,
~1e-4 relative error).
"""

import math

import numpy as np

import concourse.bass as bass
import concourse.mybir as mybir
from concourse import bacc
from concourse.bass_utils import run_bass_kernel_spmd
from concourse.tile import TileContext

dt = mybir.dt
F32 = dt.float32
F32R = dt.float32r
AF = mybir.ActivationFunctionType

D_MODEL = 2048
N_HEADS = 16
D_HEAD = 128
B = 2
T = 2048
N_CORES = 8
HG = 4          # heads per core
TB = 512        # token block (matmul free dim)
NTB = T // TB   # 4
NKT = T // 128  # 16 k-tiles per head


PHASES = "123"  # profiling aid: which phases to emit


def _emit_iteration(nc, tc, it, tensors, consts):
    (xT, wT, wpT, out, dpool, cc, ss) = tensors
    (gains_sb, masks_sb, ones_sb) = consts

    qk_dram = dpool.tile([2 * HG * D_HEAD, T], F32R, tag="qk", name=f"qk{it}")
    v_dram = dpool.tile([T, HG * D_HEAD], F32R, tag="v", name=f"v{it}")

    # ---------------- Phase 1: QKV projection + RoPE ----------------
    with tc.tile_pool(name="wq", bufs=NKT) as wpool, \
         tc.tile_pool(name="xp", bufs=26) as xpool, \
         tc.tile_pool(name="cs1", bufs=1) as cspool, \
         tc.tile_pool(name="st1", bufs=4) as st1, \
         tc.tile_pool(name="psA", bufs=4, space="PSUM") as psA, \
         tc.tile_pool(name="psB", bufs=4, space="PSUM") as psB:
        cc_sb = cspool.tile([128, T], F32, tag="cc", name=f"cc{it}")
        nc.scalar.dma_start(out=cc_sb[:], in_=cc[:])
        ss_sb = cspool.tile([128, T], F32, tag="ss", name=f"ss{it}")
        nc.scalar.dma_start(out=ss_sb[:], in_=ss[:])
        # W loads split by q/k/v column thirds, arriving in consumption
        # order (x0+wq interleaved, then wk, then wv) so the tb=0
        # accumulations ramp with the DMA stream instead of stalling on
        # the full 12MB weight preload
        w_tiles = []
        x_tiles0 = []
        for cb in range(NKT):
            w_t = wpool.tile([128, 3 * HG * D_HEAD], F32R, tag="w",
                             name=f"w{it}_{cb}")
            w_tiles.append(w_t)
            x_t = xpool.tile([128, TB], F32R, tag="x", name=f"x{it}_0_{cb}")
            nc.sync.dma_start(out=x_t[:], in_=xT[cb * 128:(cb + 1) * 128, 0:TB])
            x_tiles0.append(x_t)
            nc.sync.dma_start(out=w_t[:, 0:512],
                              in_=wT[cb * 128:(cb + 1) * 128, 0:512])
        for cb in range(NKT):
            nc.sync.dma_start(out=w_tiles[cb][:, 512:1024],
                              in_=wT[cb * 128:(cb + 1) * 128, 512:1024])
        for cb in range(NKT):
            nc.sync.dma_start(out=w_tiles[cb][:, 1024:1536],
                              in_=wT[cb * 128:(cb + 1) * 128, 1024:1536])

        for tb in range(NTB):
            tsl = slice(tb * TB, (tb + 1) * TB)
            if tb == 0:
                x_tiles = x_tiles0
            else:
                x_tiles = []
                for cb in range(NKT):
                    x_t = xpool.tile([128, TB], F32R, tag="x", name=f"x{it}_{tb}_{cb}")
                    nc.sync.dma_start(out=x_t[:], in_=xT[cb * 128:(cb + 1) * 128, tsl])
                    x_tiles.append(x_t)

            # q (m=0..3) and k (m=4..7) head-dim tiles, [d, t] layout
            for m in range(2 * HG):
                q_ps = psA.tile([128, TB], F32, tag="qps", name="q_ps")
                for cb in range(NKT):
                    nc.tensor.matmul(
                        q_ps[:], w_tiles[cb][:, m * 128:(m + 1) * 128],
                        x_tiles[cb][:],
                        start=(cb == 0), stop=(cb == NKT - 1))
                # RoPE: rows 0:64 pair with rows 64:128
                qc = st1.tile([128, TB], F32, tag="qc", name="qc")
                nc.vector.tensor_mul(qc[:], q_ps[:], cc_sb[:, tsl])
                qs = psB.tile([128, TB], F32, tag="qs", name="qs")
                nc.vector.tensor_mul(qs[:], q_ps[:], ss_sb[:, tsl])
                qr = st1.tile([128, TB], F32R, tag="qr", name="qr")
                nc.vector.tensor_sub(qr[0:64, :], qc[0:64, :], qs[64:128, :])
                nc.vector.tensor_add(qr[64:128, :], qc[64:128, :], qs[0:64, :])
                nc.sync.dma_start(out=qk_dram[m * 128:(m + 1) * 128, tsl],
                                  in_=qr[:])

            # v in [t, d] layout: x tiles stationary, wv moving
            for tsub in range(TB // 128):
                v_ps = psA.tile([128, HG * D_HEAD], F32, tag="qps", name="v_ps")
                for cb in range(NKT):
                    nc.tensor.matmul(
                        v_ps[:],
                        x_tiles[cb][:, tsub * 128:(tsub + 1) * 128],
                        w_tiles[cb][:, 2 * HG * D_HEAD:3 * HG * D_HEAD],
                        start=(cb == 0), stop=(cb == NKT - 1))
                vr = st1.tile([128, HG * D_HEAD], F32R, tag="vr", name="vr")
                nc.scalar.copy(vr[:], v_ps[:])
                r0 = tb * TB + tsub * 128
                nc.sync.dma_start(out=v_dram[r0:r0 + 128, :], in_=vr[:])

    if "2" not in PHASES:
        return
    # ---------------- Phase 2: attention, Phase 3: projection -------
    with tc.tile_pool(name="yt", bufs=1) as ytpool, \
         tc.tile_pool(name="wp", bufs=1) as wppool:

        wp_tiles = []
        for h in range(HG):
            wp_t = wppool.tile([128, D_MODEL], F32R, tag=f"wp{h}",
                               name=f"wp{it}_{h}")
            nc.sync.dma_start(out=wp_t[:], in_=wpT[h * 128:(h + 1) * 128, :])
            wp_tiles.append(wp_t)

        yT_tiles = []
        for h in range(HG):
            yT_tiles.append(ytpool.tile([128, T], F32R, tag=f"yT{h}",
                                        name=f"yT{it}_{h}"))

        _phase2(nc, tc, it, qk_dram, v_dram, gains_sb, masks_sb, ones_sb,
                yT_tiles)
        _phase3(nc, tc, it, out, yT_tiles, wp_tiles)


def _phase2(nc, tc, it, qk_dram, v_dram, gains_sb, masks_sb, ones_sb,
            yT_tiles):
    with tc.tile_pool(name="qk2", bufs=2) as qkpool, \
         tc.tile_pool(name="vt2", bufs=2) as vtpool, \
         tc.tile_pool(name="ep", bufs=6) as epool, \
         tc.tile_pool(name="sm", bufs=2) as smpool, \
         tc.tile_pool(name="psS", bufs=2, space="PSUM") as psS, \
         tc.tile_pool(name="psY", bufs=3, space="PSUM") as psY, \
         tc.tile_pool(name="psD", bufs=1, space="PSUM") as psD:
        for h in range(HG):
            qt = qkpool.tile([128, T], F32R, tag="qt", name="qt")
            kt = qkpool.tile([128, T], F32R, tag="kt", name="kt")
            vt = vtpool.tile([128, T], F32R, tag="vt", name="vt")
            # split loads per t-block so phase 2 can start as soon as
            # phase 1's early t-blocks hit DRAM
            # loads issued from the (otherwise idle) gpsimd queue so they
            # prefetch during phase 1 as soon as the scratch writes land
            for tb in range(NTB):
                tsl = slice(tb * TB, (tb + 1) * TB)
                nc.gpsimd.dma_start(out=qt[:, tsl],
                                    in_=qk_dram[h * 128:(h + 1) * 128, tsl])
                nc.gpsimd.dma_start(
                    out=kt[:, tsl],
                    in_=qk_dram[(HG + h) * 128:(HG + h + 1) * 128, tsl])
            nc.gpsimd.dma_start(
                out=vt[:].rearrange("p (a d) -> p a d", a=NKT),
                in_=v_dram[:, h * 128:(h + 1) * 128].rearrange(
                    "(a p) d -> p a d", p=128))

            for qb in range(NTB):
                qsl = slice(qb * TB, (qb + 1) * TB)
                y_ps = psY.tile([128, TB], F32, tag="y", name="y_ps")
                d_ps = psD.tile([1, TB], F32, tag="d", name="d_ps")
                nkb = 4 * qb + 4
                # kb pairs: one [128, 1024] psum -> single wide exp on ACT
                for p in range(nkb // 2):
                    kb0, kb1 = 2 * p, 2 * p + 1
                    s_ps = psS.tile([128, 2 * TB], F32, tag="s", name="s_ps")
                    nc.tensor.matmul(s_ps[:, 0:TB],
                                     kt[:, kb0 * 128:(kb0 + 1) * 128],
                                     qt[:, qsl], start=True, stop=True)
                    nc.tensor.matmul(s_ps[:, TB:2 * TB],
                                     kt[:, kb1 * 128:(kb1 + 1) * 128],
                                     qt[:, qsl], start=True, stop=True)
                    e_sb = epool.tile([128, 2 * TB], F32R, tag="e", name="e_sb")
                    nc.scalar.activation(e_sb[:], s_ps[:], AF.Exp,
                                         scale=gains_sb[:, h:h + 1])
                    r0 = 2 * p - 4 * qb
                    if r0 >= 0:  # diagonal region: causal mask (r0, r0+1)
                        nc.vector.tensor_mul(
                            e_sb[:], e_sb[:],
                            masks_sb[:, r0 * TB:(r0 + 2) * TB])
                    for j, kb in enumerate((kb0, kb1)):
                        esl = slice(j * TB, (j + 1) * TB)
                        nc.tensor.matmul(y_ps[:],
                                         vt[:, kb * 128:(kb + 1) * 128],
                                         e_sb[:, esl],
                                         start=(kb == 0), stop=(kb == nkb - 1))
                        nc.tensor.matmul(d_ps[:], ones_sb[:], e_sb[:, esl],
                                         start=(kb == 0), stop=(kb == nkb - 1))
                rec = smpool.tile([1, TB], F32, tag="rec", name="rec")
                nc.vector.reciprocal(rec[:], d_ps[:])
                recb = smpool.tile([128, TB], F32, tag="recb", name="recb")
                nc.gpsimd.partition_broadcast(recb[:], rec[:])
                nc.vector.tensor_mul(yT_tiles[h][:, qsl], y_ps[:], recb[:])


def _phase3(nc, tc, it, out, yT_tiles, wp_tiles):
    # out[t, o] partial = sum_hd yT[hd, t] * wpT[hd, o]
    with tc.tile_pool(name="ost", bufs=6) as ostage, \
         tc.tile_pool(name="psP", bufs=6, space="PSUM") as psP:
        for mt in range(NKT if "3" in PHASES else 0):
            msl = slice(mt * 128, (mt + 1) * 128)
            for ob in range(NTB):
                osl = slice(ob * TB, (ob + 1) * TB)
                p_ps = psP.tile([128, TB], F32, tag="p", name="p_ps")
                for h in range(HG):
                    nc.tensor.matmul(p_ps[:], yT_tiles[h][:, msl],
                                     wp_tiles[h][:, osl],
                                     start=(h == 0), stop=(h == HG - 1))
                o_sb = ostage.tile([128, TB], F32, tag="o", name="o_sb")
                if (mt + ob) % 2 == 0:
                    nc.scalar.copy(o_sb[:], p_ps[:])
                else:
                    nc.vector.tensor_copy(o_sb[:], p_ps[:])
                nc.sync.dma_start(out=out[msl, osl], in_=o_sb[:])


def build_program(repeat=1):
    nc = bacc.Bacc("TRN2", target_bir_lowering=False, debug=False,
                   num_devices=N_CORES)

    xT = nc.dram_tensor("xT", [D_MODEL, T], F32R, kind="ExternalInput")
    wT = nc.dram_tensor("wT", [D_MODEL, 3 * HG * D_HEAD], F32R, kind="ExternalInput")
    wpT = nc.dram_tensor("wpT", [HG * D_HEAD, D_MODEL], F32R, kind="ExternalInput")
    cc = nc.dram_tensor("cc", [128, T], F32, kind="ExternalInput")
    ss = nc.dram_tensor("ss", [128, T], F32, kind="ExternalInput")
    gains = nc.dram_tensor("gains", [128, HG], F32, kind="ExternalInput")
    masks = nc.dram_tensor("masks", [128, 4 * TB], F32R, kind="ExternalInput")
    ones = nc.dram_tensor("ones", [128, 1], F32R, kind="ExternalInput")
    out = nc.dram_tensor("out", [T, D_MODEL], F32, kind="ExternalOutput")

    with TileContext(nc) as tc:
        with tc.tile_pool(name="const", bufs=1) as cpool, \
             tc.tile_pool(name="dram", bufs=1, space="DRAM") as dpool:
            gains_sb = cpool.tile([128, HG], F32, tag="gains")
            nc.scalar.dma_start(out=gains_sb[:], in_=gains[:])
            masks_sb = cpool.tile([128, 4 * TB], F32R, tag="masks")
            nc.scalar.dma_start(out=masks_sb[:], in_=masks[:])
            ones_sb = cpool.tile([128, 1], F32R, tag="ones")
            nc.scalar.dma_start(out=ones_sb[:], in_=ones[:])

            tensors = (xT, wT, wpT, out, dpool, cc, ss)
            consts = (gains_sb, masks_sb, ones_sb)
            for it in range(repeat):
                _emit_iteration(nc, tc, it, tensors, consts)

    nc.compile()
    return nc


def prepare_core_inputs(x, Wqkv, Wproj, q_gain, rope_cos, rope_sin):
    x = np.asarray(x, dtype=np.float32)
    Wqkv = np.asarray(Wqkv, dtype=np.float32)
    Wproj = np.asarray(Wproj, dtype=np.float32)
    q_gain = np.asarray(q_gain, dtype=np.float32)
    rope_cos = np.asarray(rope_cos, dtype=np.float32)
    rope_sin = np.asarray(rope_sin, dtype=np.float32)

    cosT = np.ascontiguousarray(rope_cos.T)  # [64, T]
    sinT = np.ascontiguousarray(rope_sin.T)
    cc = np.concatenate([cosT, cosT], axis=0)  # [128, T]
    ss = np.concatenate([sinT, sinT], axis=0)

    masks = np.zeros((128, 4 * TB), dtype=np.float32)
    kk = np.arange(128)[:, None]
    qq = np.arange(TB)[None, :]
    for r in range(4):
        masks[:, r * TB:(r + 1) * TB] = ((r * 128 + kk) <= qq)

    ones = np.ones((128, 1), dtype=np.float32)
    scale = 1.0 / math.sqrt(D_HEAD)

    xT_b = [np.ascontiguousarray(x[b].T) for b in range(B)]  # [C, T]
    in_maps = []
    for c in range(N_CORES):
        b = c // HG
        hg = c % HG
        r0 = 512 * hg
        wsel = np.concatenate([
            Wqkv[r0:r0 + 512],                       # q rows for 4 heads
            Wqkv[D_MODEL + r0:D_MODEL + r0 + 512],   # k rows
            Wqkv[2 * D_MODEL + r0:2 * D_MODEL + r0 + 512],  # v rows
        ], axis=0)                                    # [1536, C]
        wT = np.ascontiguousarray(wsel.T)             # [C, 1536]
        wpT = np.ascontiguousarray(Wproj[:, r0:r0 + 512].T)  # [512, C]
        g = (q_gain[4 * hg:4 * hg + 4] * scale).astype(np.float32)
        gains = np.ascontiguousarray(np.broadcast_to(g[None, :], (128, HG)))
        in_maps.append({
            "xT": xT_b[b], "wT": wT, "wpT": wpT, "cc": cc, "ss": ss,
            "gains": gains, "masks": masks, "ones": ones,
        })
    return in_maps


_NC_CACHE = []


def kernel(x, Wqkv, Wproj, q_gain, rope_cos, rope_sin):
    if not _NC_CACHE:
        _NC_CACHE.append(build_program())
    nc = _NC_CACHE[0]
    in_maps = prepare_core_inputs(x, Wqkv, Wproj, q_gain, rope_cos, rope_sin)
    res = run_bass_kernel_spmd(nc, in_maps, list(range(N_CORES)))
    out = np.zeros((B, T, D_MODEL), dtype=np.float32)
    for c in range(N_CORES):
        out[c // HG] += res.results[c]["out"]
    return out
